# revision 13
# baseline (speedup 1.0000x reference)
"""Bass/Tile kernel for nn_BitDanceFP8ScaledLinear (column-parallel over 8 NeuronCores).

y = x @ (weight * weight_scale[:, None]).T + bias
  x: [4, 2048, 4096] f32, weight: [11008, 4096] f32, weight_scale/bias: [11008] f32

Strategy (per core c of 8):
  - weight/scale/bias sharded along out_features (1376 per core); x replicated.
  - Host-side (lossless layout prep only): x flattened+transposed to xT [4096, 8192];
    weight shard transposed to wT [4096, 1376]; scale/bias replicated to [128, 1376].
  - Device: wT and xT are DMA-loaded with an inline fp32->bf16 cast (SWDGE round-to-
    nearest), matmuls run in bf16 at full PE rate accumulating fp32 in PSUM
    (psum[tokens=128, outF<=512] += xT_tile.T @ wT_tile over 32 k-chunks),
    epilogue applies y = psum * scale + bias on the vector engine (per-column
    vectors pre-replicated across partitions), result DMA'd to y [8192, 1376] f32.
  - Host gathers: concatenate core outputs along out_features.
"""

import sys

for _p in ("/opt/trn_rl_repo", "/root/.axon_site/_ro/trn_rl_repo"):
    if _p not in sys.path:
        sys.path.insert(0, _p)

import numpy as np

import concourse.tile as tile
from concourse.tile import add_dep_helper
from concourse import bacc, bass_utils, mybir

B, S, IN, OUT = 4, 2048, 4096, 11008
N_CORES = 8
OUT_SH = OUT // N_CORES  # 1376
TOKENS = B * S  # 8192
P = 128
KO = IN // P  # 32 contraction chunks
T_BLK = 256  # tokens per x load block
N_SPLITS = [(0, 512), (512, 512), (1024, 352)]  # OUT_SH split into PSUM-bank-sized pieces

_cache = {}


def _build_program():
    nc = bacc.Bacc("TRN2", target_bir_lowering=False, debug=False, num_devices=N_CORES)

    xT = nc.dram_tensor("xT", [IN, TOKENS], mybir.dt.float32, kind="ExternalInput").ap()
    wT = nc.dram_tensor("wT", [IN, OUT_SH], mybir.dt.float32, kind="ExternalInput").ap()
    sc = nc.dram_tensor("scale_rep", [P, OUT_SH], mybir.dt.float32, kind="ExternalInput").ap()
    bi = nc.dram_tensor("bias_rep", [P, OUT_SH], mybir.dt.float32, kind="ExternalInput").ap()
    y = nc.dram_tensor("y", [TOKENS, OUT_SH], mybir.dt.float32, kind="ExternalOutput").ap()

    xT_t = xT.rearrange("(ko ki) t -> ki ko t", ki=P)  # [128, 32, 8192]
    wT_t = wT.rearrange("(ko ki) n -> ki ko n", ki=P)  # [128, 32, 1376]

    NB = T_BLK // P  # m-tiles per block

    with tile.TileContext(nc) as tc:
        with (
            tc.tile_pool(name="const", bufs=1) as const,
            tc.tile_pool(name="wstage", bufs=8) as wstage,
            tc.tile_pool(name="xp", bufs=2) as xp,
            tc.tile_pool(name="outp", bufs=4) as outp,
            tc.tile_pool(name="psum", bufs=8, space="PSUM") as psp,
        ):
            # Block-0 x first on the otherwise-empty SWDGE queue (~12us).
            xb0 = xp.tile([P, KO, T_BLK], mybir.dt.bfloat16, name="xb")
            nc.gpsimd.dma_start(xb0[:], xT_t[:, :, 0:T_BLK])

            # Weight streams n-range-major in 96 HWDGE piece loads, cast
            # f32->bf16 on DVE into persistent [P, nsz] tiles. n-range-major
            # order means each third of the stream unlocks fully-completable
            # PSUM groups, so early blocks can trail the stream densely.
            wbk = {}   # (nr, k) -> bf16 tile
            wcast = {}
            for nr, (n0, nsz) in enumerate(N_SPLITS):
                for k in range(KO):
                    wst = wstage.tile([P, 512], mybir.dt.float32, name="wst")
                    nc.sync.dma_start(wst[:, :nsz], wT_t[:, k, n0 : n0 + nsz])
                    wbt = const.tile([P, nsz], mybir.dt.bfloat16, name=f"wb_{nr}_{k}")
                    wcast[(nr, k)] = nc.vector.tensor_copy(wbt[:], wst[:, :nsz])
                    wbk[(nr, k)] = wbt

            sct = const.tile([P, OUT_SH], mybir.dt.float32)
            nc.sync.dma_start(sct[:], sc[:])
            bit = const.tile([P, OUT_SH], mybir.dt.float32)
            nc.sync.dma_start(bit[:], bi[:])

            def mm_group(ps, xb, mi, nr, interleave_with=None):
                """One PSUM accumulation group (mi, nr) over all k."""
                for k in range(KO):
                    nc.tensor.matmul(
                        ps,
                        xb[:, k, mi * P : (mi + 1) * P],
                        wbk[(nr, k)][:],
                        start=(k == 0),
                        stop=(k == KO - 1),
                    )

            def evict(ps, ot, nr):
                n0, nsz = N_SPLITS[nr]
                nc.vector.tensor_mul(ot[:, n0 : n0 + nsz], ps, sct[:, n0 : n0 + nsz])
                nc.vector.tensor_add(
                    ot[:, n0 : n0 + nsz], ot[:, n0 : n0 + nsz], bit[:, n0 : n0 + nsz]
                )

            for blk in range(TOKENS // T_BLK):
                t0 = blk * T_BLK
                if blk == 0:
                    xb = xb0
                else:
                    xb = xp.tile([P, KO, T_BLK], mybir.dt.bfloat16, name="xb")
                    xdma = nc.gpsimd.dma_start(xb[:], xT_t[:, :, t0 : t0 + T_BLK])
                    # Pace early x prefetches behind the one-time weight
                    # stream so the SDMA round-robin doesn't starve it.
                    gate = {1: (0, 20), 2: (1, 31), 3: (2, 31)}.get(blk)
                    if gate is not None:
                        add_dep_helper(xdma.ins, wcast[gate].ins, sync=True,
                                       reason="pace x prefetch behind w stream")

                if blk <= 1:
                    # n-range-major group order, k-interleaved across the two
                    # m-tiles: PE work trails the streaming weight pieces.
                    ots = [outp.tile([P, OUT_SH], mybir.dt.float32, name="ot") for _ in range(NB)]
                    for nr in range(len(N_SPLITS)):
                        nsz = N_SPLITS[nr][1]
                        pss = [psp.tile([P, 512], mybir.dt.float32, name="ps")[:, :nsz] for _ in range(NB)]
                        for k in range(KO):
                            for mi in range(NB):
                                nc.tensor.matmul(
                                    pss[mi],
                                    xb[:, k, mi * P : (mi + 1) * P],
                                    wbk[(nr, k)][:],
                                    start=(k == 0),
                                    stop=(k == KO - 1),
                                )
                        for mi in range(NB):
                            evict(pss[mi], ots[mi], nr)
                    for mi in range(NB):
                        trow = t0 + mi * P
                        nc.sync.dma_start(y[trow : trow + P, :], ots[mi][:])
                    continue

                for mi in range(NB):
                    ot = outp.tile([P, OUT_SH], mybir.dt.float32, name="ot")
                    for nr in range(len(N_SPLITS)):
                        nsz = N_SPLITS[nr][1]
                        ps_full = psp.tile([P, 512], mybir.dt.float32, name="ps")
                        ps = ps_full[:, :nsz]
                        mm_group(ps, xb, mi, nr)
                        evict(ps, ot, nr)
                    trow = t0 + mi * P
                    nc.sync.dma_start(y[trow : trow + P, :], ot[:])

    nc.compile()
    return nc


def _prep_inputs(x, weight, weight_scale, bias):
    x2 = np.ascontiguousarray(x, dtype=np.float32).reshape(TOKENS, IN)
    xT = np.ascontiguousarray(x2.T)  # [4096, 8192], shared across cores
    in_maps = []
    for c in range(N_CORES):
        lo, hi = c * OUT_SH, (c + 1) * OUT_SH
        wTc = np.ascontiguousarray(weight[lo:hi, :].astype(np.float32, copy=False).T)
        scc = np.ascontiguousarray(
            np.broadcast_to(weight_scale[lo:hi].astype(np.float32, copy=False)[None, :], (P, OUT_SH))
        )
        bic = np.ascontiguousarray(
            np.broadcast_to(bias[lo:hi].astype(np.float32, copy=False)[None, :], (P, OUT_SH))
        )
        in_maps.append({"xT": xT, "wT": wTc, "scale_rep": scc, "bias_rep": bic})
    return in_maps


def kernel(x, weight, weight_scale, bias, _trace=False):
    if "nc" not in _cache:
        _cache["nc"] = _build_program()
    nc = _cache["nc"]
    in_maps = _prep_inputs(x, weight, weight_scale, bias)
    res = bass_utils.run_bass_kernel_spmd(
        nc, in_maps, core_ids=list(range(N_CORES)), trace=_trace
    )
    _cache["last_result"] = res
    out = np.concatenate([res.results[c]["y"] for c in range(N_CORES)], axis=1)
    return out.reshape(B, S, OUT)
